# revision 26
# baseline (speedup 1.0000x reference)
"""Trainium2 Bass kernel for nn_Doppler_operatorV2.

Computes out = reshape_F(pre @ A_w.T, (8,4,96,96)) where pre is the
wrapped-finite-difference preprocess of x (see reference).

Strategy (8 cores, SPMD, no collectives):
  - Every core computes the full preprocess pre^T (9216 x 32) redundantly
    (tiny: ~1.2 MB of activations, a few 96x3072 matmuls).
  - The dense operator is column-sharded: core k computes
    y[:, 1152k:1152(k+1)] = pre @ A_w[:, cols]  (A_w is exactly symmetric,
    so A_w.T == A_w bitwise and row-major tiles contract naturally).
  - Host slices A_w columns per core, gathers the 8 output slices.

The environment's jax patches `%` on arrays to
  a % b = a - round_afz((a - (b-1)/2)/b) * b
so wrap(v) = (v+1)%2 - 1 is replicated bit-exactly on DVE via the
round-to-even-multiple magic constant plus a 2^-23 tie-break bias
(ties round away from zero in jax; the bias is far below the data's
margin to the rounding boundaries, so only exact ties are affected).
All pre-wrap matmuls have 2-nonzero rows (difference matrices), so they
are order-independent and bitwise reproducible on the PE.
"""

import numpy as np

import concourse.bass as bass
import concourse.tile as tile
from concourse import mybir
from concourse.bass_utils import run_bass_kernel_spmd

F32 = mybir.dt.float32
F32R = mybir.dt.float32r
ALU = mybir.AluOpType

MAGIC = 1.5 * 2**24   # rounds to even multiples of 2 at fp32
TIE_BIAS = 2.0**-23   # breaks round ties away from zero (positive side)

N_CORES = 8
CW = 9216 // N_CORES  # 1152 output columns per core
NK = 72               # 9216 / 128 contraction tiles
GEMM_N = 384          # moving free-dim per matmul (3 chunks of 1152)


# --- workaround: walrus CTRL encoding only carries 1 sem wait; the
# TileContext end-of-context Drain waits on one sem per engine/DMA lane.
# Split the waits across a chain of Drains.
def _apply_tile_drain_patch():
    import bass_rust
    from concourse.vector_clock import ScopedClock

    def _patched(self, tick_clock, wait_clock):
        drain_inst = self.nc.sync.drain()
        wait_clock.add_sem_waits(
            drain_inst.ins, ScopedClock({None: tick_clock.global_clock})
        )
        ins = drain_inst.ins
        si = ins.sync_info
        if si is not None and si.on_wait is not None and len(si.on_wait) > 1:
            waits = list(si.on_wait)
            ins.sync_info = bass_rust.SyncInfo(
                on_wait=waits[:1], on_update=si.on_update
            )
            for i in range(1, len(waits)):
                extra = self.nc.sync.drain()
                esi = extra.ins.sync_info
                extra.ins.sync_info = bass_rust.SyncInfo(
                    on_wait=waits[i : i + 1],
                    on_update=esi.on_update if esi is not None else [],
                )
        self.nc.all_engine_barrier()
        assert self.sems is not None
        popped = self.nc._tile_sem_poison_stack.pop()
        assert popped is self._sem_poison
        self.nc.clear_and_free_semaphores(list(self.sems.allocated().values()))
        self.nc.all_engine_barrier()

    tile.TileContext._drain_and_barrier = _patched


_apply_tile_drain_patch()


def _split_excess_waits(nc, maxw=1):
    """Walrus codegen only encodes `maxw` sem waits per instruction in this
    toolchain. Hoist excess waits onto nofuse nops inserted just before the
    offending instruction (same engine, same program order => same gating)."""
    import bass_rust

    for fn in nc.m.functions:
        for bb in fn.blocks:
            insts = bb.instructions
            new_list = []
            changed = False
            for inst in insts:
                si = getattr(inst, "sync_info", None)
                ow = si.on_wait if si is not None else None
                if ow and len(ow) > maxw:
                    changed = True
                    waits = list(ow)
                    for w in waits[:-maxw]:
                        nop = nc.engines[inst.engine].nop(nofuse=True).ins
                        for bb2 in fn.blocks:
                            if bb2.instructions and bb2.instructions[-1] is nop:
                                bb2.instructions.pop()
                                break
                        nop.sync_info = bass_rust.SyncInfo(
                            on_wait=[w], on_update=[]
                        )
                        new_list.append(nop)
                    inst.sync_info = bass_rust.SyncInfo(
                        on_wait=waits[-maxw:], on_update=si.on_update
                    )
                new_list.append(inst)
            if changed:
                insts[:] = new_list


def _emit_wrap(nc, pool, dst, ps, engine="dve"):
    """dst = wrap(ps) replicating the env's patched fp32 modulo.

    a = ps + 1; d = (a - 0.5) + 2^-23; f2 = (d + MAGIC) - MAGIC;
    dst = (a - 1) - f2.  The affine chain runs on DVE (fused 2-op
    tensor_scalars) or ACT (bias-adds; verified bit-exact on HW); the final
    two-tensor pass is always DVE.
    """
    AFT = mybir.ActivationFunctionType
    p, f = ps.shape
    a_t = pool.tile([p, f], F32, tag=f"wrap_a_{engine}")
    f_t = pool.tile([p, f], F32, tag=f"wrap_f_{engine}")
    if engine == "dve":
        nc.vector.tensor_scalar(a_t[:], ps[:], 1.0, None, ALU.add)
        d_t = pool.tile([p, f], F32, tag="wrap_d")
        nc.vector.tensor_scalar(
            d_t[:], a_t[:], 0.5, TIE_BIAS, ALU.subtract, ALU.add
        )
        nc.vector.tensor_scalar(
            f_t[:], d_t[:], MAGIC, MAGIC, ALU.add, ALU.subtract
        )
    else:
        # two separate adds (-0.5 then +TIE_BIAS) replicate jax's rounding
        # sequence bit-exactly; a folded constant would not.
        nc.scalar.activation(a_t[:], ps[:], AFT.Copy, bias=1.0, scale=1.0)
        d2_t = pool.tile([p, f], F32, tag="wrap_d2a")
        nc.scalar.activation(d2_t[:], a_t[:], AFT.Copy, bias=-0.5, scale=1.0)
        d1_t = pool.tile([p, f], F32, tag="wrap_d1a")
        nc.scalar.activation(d1_t[:], d2_t[:], AFT.Copy, bias=TIE_BIAS, scale=1.0)
        f1_t = pool.tile([p, f], F32, tag="wrap_f1a")
        nc.scalar.activation(f1_t[:], d1_t[:], AFT.Copy, bias=MAGIC, scale=1.0)
        nc.scalar.activation(f_t[:], f1_t[:], AFT.Copy, bias=-MAGIC, scale=1.0)
    nc.vector.scalar_tensor_tensor(
        dst, a_t[:], 1.0, f_t[:], ALU.subtract, ALU.subtract
    )


def build_kernel():
    nc = bass.Bass()

    x1_d = nc.dram_tensor("X1", [96, 3072], F32, kind="ExternalInput")
    x2_d = nc.dram_tensor("X2", [96, 3072], F32, kind="ExternalInput")
    xs_d = nc.dram_tensor("Xs", [96, 3072], F32, kind="ExternalInput")
    dm_d = nc.dram_tensor("DMw", [96, 96], F32, kind="ExternalInput")
    dmt_d = nc.dram_tensor("DMT", [96, 96], F32, kind="ExternalInput")
    dn_d = nc.dram_tensor("DNw", [96, 96], F32, kind="ExternalInput")
    dnt_d = nc.dram_tensor("DNT", [96, 96], F32, kind="ExternalInput")
    eta_d = nc.dram_tensor("eta96", [96, 1], F32, kind="ExternalInput")
    a_d = nc.dram_tensor("A", [9216, CW], F32, kind="ExternalInput")
    y_d = nc.dram_tensor("Y", [128, CW // 4], F32, kind="ExternalOutput")

    with tile.TileContext(nc) as tc:
        with (
            tc.tile_pool(name="const", bufs=1) as cpool,
            tc.tile_pool(name="acts", bufs=1) as apool,
            tc.tile_pool(name="wrapt", bufs=2) as wpool,
            tc.tile_pool(name="astream", bufs=16) as spool,
            tc.tile_pool(name="pp", bufs=2, space="PSUM") as pp,
            tc.tile_pool(name="pss", bufs=2, space="PSUM") as pss,
            tc.tile_pool(name="py", bufs=1, space="PSUM") as py,
            tc.tile_pool(name="pwarm", bufs=1, space="PSUM") as pw,
            tc.tile_pool(name="scratch", bufs=1, space="DRAM") as dpool,
        ):
            # ---- PE warmup: HAM unthrottles after ~3.4us of activity ----
            junk = cpool.tile([128, 384], F32, tag="junk")
            nc.gpsimd.memset(junk[:], 0.0)
            psw = pw.tile([128, 256], F32, tag="psw")
            for _ in range(10):
                nc.tensor.matmul(
                    psw[:], junk[:, 0:128], junk[:, 128:384],
                    start=True, stop=True,
                )

            # ---- constants & activations in ----
            # x-branch inputs first: its chain (wrap -> shuffle -> term2) is
            # the critical path to the big GEMM.
            dnt_t = cpool.tile([96, 96], F32, tag="dnt")
            nc.scalar.dma_start(dnt_t[:], dnt_d.ap())
            x2_t = apool.tile([96, 3072], F32, tag="bigA")
            nc.scalar.dma_start(x2_t[:], x2_d.ap())
            dm_t = cpool.tile([96, 96], F32, tag="dm")
            dmt_t = cpool.tile([96, 96], F32, tag="dmt")
            dn_t = cpool.tile([96, 96], F32, tag="dn")
            eta_t = cpool.tile([96, 1], F32, tag="eta")
            nc.scalar.dma_start(dn_t[:], dn_d.ap())
            nc.scalar.dma_start(dm_t[:], dm_d.ap())
            nc.scalar.dma_start(dmt_t[:], dmt_d.ap())
            nc.scalar.dma_start(eta_t[:], eta_d.ap())
            x1_t = apool.tile([96, 3072], F32, tag="bigB")
            xs_t = apool.tile([96, 3072], F32, tag="bigC")
            nc.scalar.dma_start(x1_t[:], x1_d.ap())
            nc.scalar.dma_start(xs_t[:], xs_d.ap())

            w1s = apool.tile([96, 3072], F32, tag="bigD")
            w2ts = apool.tile([96, 3072], F32, tag="w2ts")
            va = apool.tile([96, 3072], F32, tag="va")
            x2all = apool.tile([96, 3072], F32, tag="bigA")
            pret = apool.tile([128, NK * 32], F32, tag="pret", bufs=1)
            ysb = apool.tile([128, CW // 4], F32, tag="ysb", bufs=1)

            # ---- branch 2 (critical): W2T = wrap(DN @ X2) ----
            for j in range(6):
                ps = pp.tile([96, 512], F32, tag="pp")
                nc.tensor.matmul(
                    ps[:], dnt_t[:], x2_t[:, 512 * j : 512 * (j + 1)],
                    start=True, stop=True,
                )
                _emit_wrap(nc, wpool, w2ts[:, 512 * j : 512 * (j + 1)], ps)

            # ---- shuffle: x2all[32i+ss, 96s+m] = w2ts[3s+i, 96ss+m] ----
            # Non-affine across partitions; linearize through a DRAM scratch:
            # hop1 scatters w2ts into ldram[(i,ss), (s,m)] order, hop2 loads
            # it back contiguously with q3 on partitions.
            ldram = dpool.tile([96, 3072], F32, tag="ldram")
            w2ts_r = w2ts[:].rearrange("(s i) (ss m) -> s i ss m", i=3, m=96)
            ldram_r = ldram[:].rearrange("(i ss) (s m) -> i s ss m", ss=32, m=96)
            for i in range(3):
                for gg in range(2):
                    nc.scalar.dma_start(
                        ldram_r[i, :, 16 * gg : 16 * (gg + 1)],
                        w2ts_r[:, i, 16 * gg : 16 * (gg + 1)],
                    )
            # hop2 in 4 column chunks (8 s-slices each) so term2 starts early
            for gg in range(4):
                nc.scalar.dma_start(
                    x2all[:, 768 * gg : 768 * (gg + 1)],
                    ldram[:, 768 * gg : 768 * (gg + 1)],
                )

            # ---- branch 1: W1 = wrap(DM @ X1) ----
            # split the affine wrap chain across ACT and DVE to halve the
            # DVE serial bottleneck
            for j in range(6):
                ps = pp.tile([96, 512], F32, tag="pp")
                nc.tensor.matmul(
                    ps[:], dmt_t[:], x1_t[:, 512 * j : 512 * (j + 1)],
                    start=True, stop=True,
                )
                _emit_wrap(nc, wpool, w1s[:, 512 * j : 512 * (j + 1)], ps)

            # ---- Va = DM.T @ W1 + eta * Xs ----
            for j in range(6):
                ps = pp.tile([96, 512], F32, tag="pp")
                nc.tensor.matmul(
                    ps[:], dm_t[:], w1s[:, 512 * j : 512 * (j + 1)],
                    start=True, stop=True,
                )
                nc.vector.scalar_tensor_tensor(
                    va[:, 512 * j : 512 * (j + 1)],
                    xs_t[:, 512 * j : 512 * (j + 1)],
                    eta_t[:, 0:1],
                    ps[:],
                    ALU.mult,
                    ALU.add,
                )

            # ---- term2: Va[:, 96s:96s+96] += X2all_s.T @ DN ----
            # group 5 s-slices per PSUM bank; one DVE add per group
            for g in range(7):
                s_lo = 5 * g
                s_hi = min(s_lo + 5, 32)
                gw = 96 * (s_hi - s_lo)
                ps2 = pss.tile([96, 480], F32, tag="pss")
                for s2 in range(s_lo, s_hi):
                    off = 96 * (s2 - s_lo)
                    nc.tensor.matmul(
                        ps2[:, off : off + 96],
                        x2all[:, 96 * s2 : 96 * s2 + 96],
                        dn_t[:],
                        start=True, stop=True,
                    )
                nc.vector.tensor_tensor(
                    va[:, 96 * s_lo : 96 * s_lo + gw],
                    va[:, 96 * s_lo : 96 * s_lo + gw],
                    ps2[:, 0:gw],
                    ALU.add,
                )

            # ---- preT materialization: pret[(96n2+m)%128, 32t+s] = va[m, 96s+n2]
            # r = 96*n2 + m is affine in (n2, m) in flat DRAM but not across
            # SBUF partitions; linearize through DRAM: hop1 writes
            # pdram[r, s] = va[m, 96s+n2], hop2 reloads with r%128 on
            # partitions.
            # 12 chunks of 8 n2-blocks (= 6 pret k-tiles each) so the GEMM can
            # start as soon as the first chunk lands.
            vb = apool.tile([96, 3072], F32, tag="bigD")
            nc.vector.tensor_copy(
                vb[:].rearrange("m (n s) -> m n s", s=32),
                va[:].rearrange("m (s n) -> m n s", n=96),
            )
            for cch in range(12):
                pdram_c = dpool.tile(
                    [768, 32], F32, tag=f"pdram{cch}", name=f"pdram{cch}"
                )
                nc.scalar.dma_start(
                    pdram_c[:].rearrange("(n m) s -> m n s", m=96),
                    vb[:, 256 * cch : 256 * (cch + 1)].rearrange(
                        "m (n s) -> m n s", s=32
                    ),
                )
                nc.scalar.dma_start(
                    pret[:, 192 * cch : 192 * (cch + 1)].rearrange(
                        "p (t s) -> p t s", s=32
                    ),
                    pdram_c[:].rearrange("(t p) s -> p t s", p=128),
                )

            # ---- big GEMM: y[s, c] = sum_r preT[r, s] * A[r, c] ----
            # fp32 matmuls stream the moving operand at 4 cyc/row; with M=32
            # only a quarter of the PE columns are used, so run four M=32
            # matmuls concurrently in the four column groups
            # (tile_position=(0, 32g)): group g covers output columns
            # [288g, 288g+288) and lands on psum partitions [32g, 32g+32).
            NCH = 288
            yp = py.tile([128, NCH], F32, tag="yacc", name="yacc")
            for t in range(NK):
                at = spool.tile([128, CW], F32, tag="at")
                nc.sync.dma_start(at[:], a_d.ap()[128 * t : 128 * (t + 1), :])
                for g in range(4):
                    nc.tensor.matmul(
                        yp[32 * g : 32 * g + 32, :],
                        pret[:, 32 * t : 32 * t + 32],
                        at[:, NCH * g : NCH * (g + 1)],
                        start=(t == 0),
                        stop=(t == NK - 1),
                        tile_position=(0, 32 * g),
                    )

            nc.vector.tensor_copy(ysb[:], yp[:])
            nc.sync.dma_start(y_d.ap(), ysb[:])

    _split_excess_waits(nc)
    return nc


_NC_CACHE = None


def _get_nc():
    global _NC_CACHE
    if _NC_CACHE is None:
        _NC_CACHE = build_kernel()
    return _NC_CACHE


def _make_in_maps(inputs):
    x = np.ascontiguousarray(np.asarray(inputs["x"], np.float32))
    A_w = np.asarray(inputs["A_w"], np.float32)
    DM = np.ascontiguousarray(np.asarray(inputs["DM"], np.float32))
    DN = np.ascontiguousarray(np.asarray(inputs["DN"], np.float32))
    eta = np.asarray(inputs["eta"], np.float32)

    X1 = np.ascontiguousarray(x.transpose(2, 3, 0, 1)).reshape(96, 3072)
    X2 = np.ascontiguousarray(x.transpose(3, 0, 1, 2)).reshape(96, 3072)
    Xs = np.ascontiguousarray(x.transpose(2, 1, 0, 3)).reshape(96, 3072)
    common = {
        "X1": X1, "X2": X2, "Xs": Xs,
        "DMw": DM, "DMT": np.ascontiguousarray(DM.T),
        "DNw": DN, "DNT": np.ascontiguousarray(DN.T),
        "eta96": np.full((96, 1), eta[0], np.float32),
    }
    return [
        dict(common, A=np.ascontiguousarray(A_w[:, CW * k : CW * (k + 1)]))
        for k in range(N_CORES)
    ]


def kernel(x, eta, A_w, DM, DN):
    in_maps = _make_in_maps(
        {"x": x, "eta": eta, "A_w": A_w, "DM": DM, "DN": DN}
    )
    nc = _get_nc()
    try:
        res = run_bass_kernel_spmd(nc, in_maps, core_ids=list(range(N_CORES)))
    except Exception:
        # transient NRT device errors recover on re-execution
        res = run_bass_kernel_spmd(nc, in_maps, core_ids=list(range(N_CORES)))

    out = np.empty((8, 4, 96, 96), np.float32)
    for k in range(N_CORES):
        yg = res.results[k]["Y"]  # [128, 288]: col group g on rows 32g:32g+32
        yk = np.concatenate(
            [yg[32 * g : 32 * g + 32, :] for g in range(4)], axis=1
        )  # [32, 1152]
        out[:, :, :, 12 * k : 12 * (k + 1)] = (
            yk.reshape(4, 8, 12, 96).transpose(1, 0, 3, 2)
        )
    return out


# revision 32
# speedup vs baseline: 1.0006x; 1.0006x over previous
"""Trainium2 Bass kernel for nn_Doppler_operatorV2.

Computes out = reshape_F(pre @ A_w.T, (8,4,96,96)) where pre is the
wrapped-finite-difference preprocess of x (see reference).

Strategy (8 cores, SPMD, no collectives):
  - Every core computes the full preprocess pre^T (9216 x 32) redundantly
    (tiny: ~1.2 MB of activations, a few 96x3072 matmuls).
  - The dense operator is column-sharded: core k computes
    y[:, 1152k:1152(k+1)] = pre @ A_w[:, cols]  (A_w is exactly symmetric,
    so A_w.T == A_w bitwise and row-major tiles contract naturally).
  - Host slices A_w columns per core, gathers the 8 output slices.

The environment's jax patches `%` on arrays to
  a % b = a - round_afz((a - (b-1)/2)/b) * b
so wrap(v) = (v+1)%2 - 1 is replicated bit-exactly on DVE via the
round-to-even-multiple magic constant plus a 2^-23 tie-break bias
(ties round away from zero in jax; the bias is far below the data's
margin to the rounding boundaries, so only exact ties are affected).
All pre-wrap matmuls have 2-nonzero rows (difference matrices), so they
are order-independent and bitwise reproducible on the PE.
"""

import numpy as np

import concourse.bass as bass
import concourse.tile as tile
from concourse import mybir
from concourse.bass_utils import run_bass_kernel_spmd

F32 = mybir.dt.float32
F32R = mybir.dt.float32r
ALU = mybir.AluOpType

MAGIC = 1.5 * 2**24   # rounds to even multiples of 2 at fp32
TIE_BIAS = 2.0**-23   # breaks round ties away from zero (positive side)

N_CORES = 8
CW = 9216 // N_CORES  # 1152 output columns per core
NK = 72               # 9216 / 128 contraction tiles
GEMM_N = 384          # moving free-dim per matmul (3 chunks of 1152)


# --- workaround: walrus CTRL encoding only carries 1 sem wait; the
# TileContext end-of-context Drain waits on one sem per engine/DMA lane.
# Split the waits across a chain of Drains.
def _apply_tile_drain_patch():
    import bass_rust
    from concourse.vector_clock import ScopedClock

    def _patched(self, tick_clock, wait_clock):
        drain_inst = self.nc.sync.drain()
        wait_clock.add_sem_waits(
            drain_inst.ins, ScopedClock({None: tick_clock.global_clock})
        )
        ins = drain_inst.ins
        si = ins.sync_info
        if si is not None and si.on_wait is not None and len(si.on_wait) > 1:
            waits = list(si.on_wait)
            ins.sync_info = bass_rust.SyncInfo(
                on_wait=waits[:1], on_update=si.on_update
            )
            for i in range(1, len(waits)):
                extra = self.nc.sync.drain()
                esi = extra.ins.sync_info
                extra.ins.sync_info = bass_rust.SyncInfo(
                    on_wait=waits[i : i + 1],
                    on_update=esi.on_update if esi is not None else [],
                )
        self.nc.all_engine_barrier()
        assert self.sems is not None
        popped = self.nc._tile_sem_poison_stack.pop()
        assert popped is self._sem_poison
        self.nc.clear_and_free_semaphores(list(self.sems.allocated().values()))
        self.nc.all_engine_barrier()

    tile.TileContext._drain_and_barrier = _patched


_apply_tile_drain_patch()


def _split_excess_waits(nc, maxw=1):
    """Walrus codegen only encodes `maxw` sem waits per instruction in this
    toolchain. Hoist excess waits onto nofuse nops inserted just before the
    offending instruction (same engine, same program order => same gating)."""
    import bass_rust

    for fn in nc.m.functions:
        for bb in fn.blocks:
            insts = bb.instructions
            new_list = []
            changed = False
            for inst in insts:
                si = getattr(inst, "sync_info", None)
                ow = si.on_wait if si is not None else None
                if ow and len(ow) > maxw:
                    changed = True
                    waits = list(ow)
                    for w in waits[:-maxw]:
                        nop = nc.engines[inst.engine].nop(nofuse=True).ins
                        for bb2 in fn.blocks:
                            if bb2.instructions and bb2.instructions[-1] is nop:
                                bb2.instructions.pop()
                                break
                        nop.sync_info = bass_rust.SyncInfo(
                            on_wait=[w], on_update=[]
                        )
                        new_list.append(nop)
                    inst.sync_info = bass_rust.SyncInfo(
                        on_wait=waits[-maxw:], on_update=si.on_update
                    )
                new_list.append(inst)
            if changed:
                insts[:] = new_list


def _emit_wrap(nc, pool, dst, ps, engine="dve"):
    """dst = wrap(ps) replicating the env's patched fp32 modulo.

    a = ps + 1; d = (a - 0.5) + 2^-23; f2 = (d + MAGIC) - MAGIC;
    dst = (a - 1) - f2.  The affine chain runs on DVE (fused 2-op
    tensor_scalars) or ACT (bias-adds; verified bit-exact on HW); the final
    two-tensor pass is always DVE.
    """
    AFT = mybir.ActivationFunctionType
    p, f = ps.shape
    a_t = pool.tile([p, f], F32, tag=f"wrap_a_{engine}")
    f_t = pool.tile([p, f], F32, tag=f"wrap_f_{engine}")
    if engine == "dve":
        nc.vector.tensor_scalar(a_t[:], ps[:], 1.0, None, ALU.add)
        d_t = pool.tile([p, f], F32, tag="wrap_d")
        nc.vector.tensor_scalar(
            d_t[:], a_t[:], 0.5, TIE_BIAS, ALU.subtract, ALU.add
        )
        nc.vector.tensor_scalar(
            f_t[:], d_t[:], MAGIC, MAGIC, ALU.add, ALU.subtract
        )
    else:
        # two separate adds (-0.5 then +TIE_BIAS) replicate jax's rounding
        # sequence bit-exactly; a folded constant would not.
        nc.scalar.activation(a_t[:], ps[:], AFT.Copy, bias=1.0, scale=1.0)
        d2_t = pool.tile([p, f], F32, tag="wrap_d2a")
        nc.scalar.activation(d2_t[:], a_t[:], AFT.Copy, bias=-0.5, scale=1.0)
        d1_t = pool.tile([p, f], F32, tag="wrap_d1a")
        nc.scalar.activation(d1_t[:], d2_t[:], AFT.Copy, bias=TIE_BIAS, scale=1.0)
        f1_t = pool.tile([p, f], F32, tag="wrap_f1a")
        nc.scalar.activation(f1_t[:], d1_t[:], AFT.Copy, bias=MAGIC, scale=1.0)
        nc.scalar.activation(f_t[:], f1_t[:], AFT.Copy, bias=-MAGIC, scale=1.0)
    nc.vector.scalar_tensor_tensor(
        dst, a_t[:], 1.0, f_t[:], ALU.subtract, ALU.subtract
    )


def build_kernel():
    nc = bass.Bass()

    x1_d = nc.dram_tensor("X1", [96, 3072], F32, kind="ExternalInput")
    x2_d = nc.dram_tensor("X2", [96, 3072], F32, kind="ExternalInput")
    xs_d = nc.dram_tensor("Xs", [96, 3072], F32, kind="ExternalInput")
    dm_d = nc.dram_tensor("DMw", [96, 96], F32, kind="ExternalInput")
    dmt_d = nc.dram_tensor("DMT", [96, 96], F32, kind="ExternalInput")
    dn_d = nc.dram_tensor("DNw", [96, 96], F32, kind="ExternalInput")
    dnt_d = nc.dram_tensor("DNT", [96, 96], F32, kind="ExternalInput")
    eta_d = nc.dram_tensor("eta96", [96, 1], F32, kind="ExternalInput")
    a_d = nc.dram_tensor("A", [9216, CW], F32, kind="ExternalInput")
    y_d = nc.dram_tensor("Y", [128, CW // 4], F32, kind="ExternalOutput")

    with tile.TileContext(nc) as tc:
        with (
            tc.tile_pool(name="const", bufs=1) as cpool,
            tc.tile_pool(name="acts", bufs=1) as apool,
            tc.tile_pool(name="wrapt", bufs=2) as wpool,
            tc.tile_pool(name="astream", bufs=8) as spool,
            tc.tile_pool(name="pp", bufs=2, space="PSUM") as pp,
            tc.tile_pool(name="pss", bufs=2, space="PSUM") as pss,
            tc.tile_pool(name="py", bufs=1, space="PSUM") as py,
            tc.tile_pool(name="pwarm", bufs=1, space="PSUM") as pw,
            tc.tile_pool(name="scratch", bufs=1, space="DRAM") as dpool,
        ):
            # ---- PE warmup: HAM unthrottles after ~3.4us of activity ----
            junk = cpool.tile([128, 384], F32, tag="junk")
            nc.gpsimd.memset(junk[:], 0.0)
            psw = pw.tile([128, 256], F32, tag="psw")
            for _ in range(6):
                nc.tensor.matmul(
                    psw[:], junk[:, 0:128], junk[:, 128:384],
                    start=True, stop=True,
                )

            # ---- constants & activations in ----
            # x-branch inputs first: its chain (wrap -> shuffle -> term2) is
            # the critical path to the big GEMM.
            dnt_t = cpool.tile([96, 96], F32, tag="dnt")
            nc.scalar.dma_start(dnt_t[:], dnt_d.ap())
            x2_t = apool.tile([96, 3072], F32, tag="bigA")
            nc.scalar.dma_start(x2_t[:], x2_d.ap())
            dm_t = cpool.tile([96, 96], F32, tag="dm")
            dmt_t = cpool.tile([96, 96], F32, tag="dmt")
            dn_t = cpool.tile([96, 96], F32, tag="dn")
            eta_t = cpool.tile([96, 1], F32, tag="eta")
            nc.scalar.dma_start(dn_t[:], dn_d.ap())
            nc.scalar.dma_start(dm_t[:], dm_d.ap())
            nc.scalar.dma_start(dmt_t[:], dmt_d.ap())
            nc.scalar.dma_start(eta_t[:], eta_d.ap())
            x1_t = apool.tile([96, 3072], F32, tag="bigB")
            xs_t = apool.tile([96, 3072], F32, tag="bigC")
            nc.scalar.dma_start(x1_t[:], x1_d.ap())
            nc.scalar.dma_start(xs_t[:], xs_d.ap())

            w1s = apool.tile([96, 3072], F32, tag="bigD")
            w2ts = apool.tile([96, 3072], F32, tag="w2ts")
            va = apool.tile([96, 3072], F32, tag="va")
            x2all = apool.tile([96, 3072], F32, tag="bigA")
            pret = apool.tile([128, NK * 32], F32, tag="pret", bufs=1)
            ysb = apool.tile([128, CW // 4], F32, tag="ysb", bufs=1)

            # ---- branch 2 (critical): W2T = wrap(DN @ X2) ----
            for j in range(6):
                ps = pp.tile([96, 512], F32, tag="pp")
                nc.tensor.matmul(
                    ps[:], dnt_t[:], x2_t[:, 512 * j : 512 * (j + 1)],
                    start=True, stop=True,
                )
                _emit_wrap(nc, wpool, w2ts[:, 512 * j : 512 * (j + 1)], ps)

            # ---- shuffle: x2all[32i+ss, 96s+m] = w2ts[3s+i, 96ss+m] ----
            # Non-affine across partitions; linearize through a DRAM scratch:
            # hop1 scatters w2ts into ldram[(i,ss), (s,m)] order, hop2 loads
            # it back contiguously with q3 on partitions.
            ldram = dpool.tile([96, 3072], F32, tag="ldram")
            w2ts_r = w2ts[:].rearrange("(s i) (ss m) -> s i ss m", i=3, m=96)
            ldram_r = ldram[:].rearrange("(i ss) (s m) -> i s ss m", ss=32, m=96)
            for i in range(3):
                for gg in range(2):
                    nc.scalar.dma_start(
                        ldram_r[i, :, 16 * gg : 16 * (gg + 1)],
                        w2ts_r[:, i, 16 * gg : 16 * (gg + 1)],
                    )
            # hop2 in 4 column chunks (8 s-slices each) so term2 starts early
            for gg in range(4):
                nc.scalar.dma_start(
                    x2all[:, 768 * gg : 768 * (gg + 1)],
                    ldram[:, 768 * gg : 768 * (gg + 1)],
                )

            # ---- branch 1: W1 = wrap(DM @ X1) ----
            # split the affine wrap chain across ACT and DVE to halve the
            # DVE serial bottleneck
            for j in range(6):
                ps = pp.tile([96, 512], F32, tag="pp")
                nc.tensor.matmul(
                    ps[:], dmt_t[:], x1_t[:, 512 * j : 512 * (j + 1)],
                    start=True, stop=True,
                )
                _emit_wrap(nc, wpool, w1s[:, 512 * j : 512 * (j + 1)], ps)

            # ---- Va = DM.T @ W1 + eta * Xs ----
            for j in range(6):
                ps = pp.tile([96, 512], F32, tag="pp")
                nc.tensor.matmul(
                    ps[:], dm_t[:], w1s[:, 512 * j : 512 * (j + 1)],
                    start=True, stop=True,
                )
                nc.vector.scalar_tensor_tensor(
                    va[:, 512 * j : 512 * (j + 1)],
                    xs_t[:, 512 * j : 512 * (j + 1)],
                    eta_t[:, 0:1],
                    ps[:],
                    ALU.mult,
                    ALU.add,
                )

            # ---- term2: Va[:, 96s:96s+96] += X2all_s.T @ DN ----
            # group 5 s-slices per PSUM bank; one DVE add per group
            for g in range(7):
                s_lo = 5 * g
                s_hi = min(s_lo + 5, 32)
                gw = 96 * (s_hi - s_lo)
                ps2 = pss.tile([96, 480], F32, tag="pss")
                for s2 in range(s_lo, s_hi):
                    off = 96 * (s2 - s_lo)
                    nc.tensor.matmul(
                        ps2[:, off : off + 96],
                        x2all[:, 96 * s2 : 96 * s2 + 96],
                        dn_t[:],
                        start=True, stop=True,
                    )
                nc.vector.tensor_tensor(
                    va[:, 96 * s_lo : 96 * s_lo + gw],
                    va[:, 96 * s_lo : 96 * s_lo + gw],
                    ps2[:, 0:gw],
                    ALU.add,
                )

            # ---- preT materialization: pret[(96n2+m)%128, 32t+s] = va[m, 96s+n2]
            # r = 96*n2 + m is affine in (n2, m) in flat DRAM but not across
            # SBUF partitions; linearize through DRAM: hop1 writes
            # pdram[r, s] = va[m, 96s+n2], hop2 reloads with r%128 on
            # partitions.
            # 12 chunks of 8 n2-blocks (= 6 pret k-tiles each) so the GEMM can
            # start as soon as the first chunk lands.
            vb = apool.tile([96, 3072], F32, tag="bigD")
            nc.vector.tensor_copy(
                vb[:].rearrange("m (n s) -> m n s", s=32),
                va[:].rearrange("m (s n) -> m n s", n=96),
            )
            for cch in range(12):
                pdram_c = dpool.tile(
                    [768, 32], F32, tag=f"pdram{cch}", name=f"pdram{cch}"
                )
                nc.scalar.dma_start(
                    pdram_c[:].rearrange("(n m) s -> m n s", m=96),
                    vb[:, 256 * cch : 256 * (cch + 1)].rearrange(
                        "m (n s) -> m n s", s=32
                    ),
                )
                nc.scalar.dma_start(
                    pret[:, 192 * cch : 192 * (cch + 1)].rearrange(
                        "p (t s) -> p t s", s=32
                    ),
                    pdram_c[:].rearrange("(t p) s -> p t s", p=128),
                )

            # ---- big GEMM: y[s, c] = sum_r preT[r, s] * A[r, c] ----
            # fp32 matmuls stream the moving operand at 4 cyc/row; with M=32
            # only a quarter of the PE columns are used, so run four M=32
            # matmuls concurrently in the four column groups
            # (tile_position=(0, 32g)): group g covers output columns
            # [288g, 288g+288) and lands on psum partitions [32g, 32g+32).
            NCH = 288
            yp = py.tile([128, NCH], F32, tag="yacc", name="yacc")
            for tp in range(NK // 2):
                # 1.18 MB per DMA (two k-tiles), partition dim outermost on
                # both sides: partition p reads rows 256tp+p and 256tp+p+128
                # as two 4.6 KB bursts.
                at = spool.tile([128, 2 * CW], F32, tag="at")
                nc.sync.dma_start(
                    at[:].rearrange("p (t c) -> p t c", t=2),
                    a_d.ap()[256 * tp : 256 * (tp + 1), :].rearrange(
                        "(t p) c -> p t c", p=128
                    ),
                )
                for t2 in range(2):
                    t = 2 * tp + t2
                    for g in range(4):
                        nc.tensor.matmul(
                            yp[32 * g : 32 * g + 32, :],
                            pret[:, 32 * t : 32 * t + 32],
                            at[:, CW * t2 + NCH * g : CW * t2 + NCH * (g + 1)],
                            start=(t == 0),
                            stop=(t == NK - 1),
                            tile_position=(0, 32 * g),
                        )

            nc.vector.tensor_copy(ysb[:], yp[:])
            nc.sync.dma_start(y_d.ap(), ysb[:])

    _split_excess_waits(nc)
    return nc


_NC_CACHE = None


def _get_nc():
    global _NC_CACHE
    if _NC_CACHE is None:
        _NC_CACHE = build_kernel()
    return _NC_CACHE


def _make_in_maps(inputs):
    x = np.ascontiguousarray(np.asarray(inputs["x"], np.float32))
    A_w = np.asarray(inputs["A_w"], np.float32)
    DM = np.ascontiguousarray(np.asarray(inputs["DM"], np.float32))
    DN = np.ascontiguousarray(np.asarray(inputs["DN"], np.float32))
    eta = np.asarray(inputs["eta"], np.float32)

    X1 = np.ascontiguousarray(x.transpose(2, 3, 0, 1)).reshape(96, 3072)
    X2 = np.ascontiguousarray(x.transpose(3, 0, 1, 2)).reshape(96, 3072)
    Xs = np.ascontiguousarray(x.transpose(2, 1, 0, 3)).reshape(96, 3072)
    common = {
        "X1": X1, "X2": X2, "Xs": Xs,
        "DMw": DM, "DMT": np.ascontiguousarray(DM.T),
        "DNw": DN, "DNT": np.ascontiguousarray(DN.T),
        "eta96": np.full((96, 1), eta[0], np.float32),
    }
    return [
        dict(common, A=np.ascontiguousarray(A_w[:, CW * k : CW * (k + 1)]))
        for k in range(N_CORES)
    ]


def kernel(x, eta, A_w, DM, DN):
    in_maps = _make_in_maps(
        {"x": x, "eta": eta, "A_w": A_w, "DM": DM, "DN": DN}
    )
    nc = _get_nc()
    try:
        res = run_bass_kernel_spmd(nc, in_maps, core_ids=list(range(N_CORES)))
    except Exception:
        # transient NRT device errors recover on re-execution
        res = run_bass_kernel_spmd(nc, in_maps, core_ids=list(range(N_CORES)))

    out = np.empty((8, 4, 96, 96), np.float32)
    for k in range(N_CORES):
        yg = res.results[k]["Y"]  # [128, 288]: col group g on rows 32g:32g+32
        yk = np.concatenate(
            [yg[32 * g : 32 * g + 32, :] for g in range(4)], axis=1
        )  # [32, 1152]
        out[:, :, :, 12 * k : 12 * (k + 1)] = (
            yk.reshape(4, 8, 12, 96).transpose(1, 0, 3, 2)
        )
    return out
